# revision 27
# baseline (speedup 1.0000x reference)
"""Trainium2 Bass kernel for CLAM_SB gated-attention MIL forward pass.

Distribution: instance dim N=100000 is sharded across 8 NeuronCores
(12500 rows each, padded to 12544 = 98*128). Each core computes its
shard of  x = relu(h @ W_fc.T + b);  a = tanh(x@Wa.T);  g = sig(x@Wb.T);
s = (a*g) @ Wc.T + bc  plus partial softmax stats  z = sum(exp(s-C)),
p = sum(exp(s-C) * x)  using a fixed host-computed shift C (softmax is
shift invariant, so no cross-core max pass is needed). The raw score
matrix (tiny, 50KB/core) plus (z, p) are returned; the host merges the
partial stats, does the 64->16 top/bottom-k merge, and evaluates the
final ~10K-FLOP classifier head.
"""

import os
import sys

import numpy as np

try:
    import concourse.bass as _probe_bass  # noqa: F401
except ImportError:
    sys.path.insert(0, "/opt/trn_rl_repo")

import concourse.bass as bass
from concourse import bacc
import concourse.mybir as mybir
from concourse.tile import TileContext
from concourse.bass_utils import run_bass_kernel_spmd

# problem constants (hardcoded per harness contract)
N_CORES = 8
N_FULL = 100000
FEAT = 1024
HID = 256
DATT = 256
K_SAMPLE = 8
TAU = 1.0
ALPHA = 1.0
N_CLASSES = 2

RV = N_FULL // N_CORES          # valid rows per core = 12500
P = 128
RT = 512                        # rows per compute tile
NT = 25                         # uniform tiles per core
R = NT * RT                     # padded rows per core = 12800
NCOL = R // P                   # 100 columns in (p, j) score layout
KT = FEAT // P                  # 8 contraction chunks for fc
MH = HID // P                   # 2 hid chunks
MD = DATT // P                  # 2 attention-dim chunks

NEG_BIG = -1.0e30

F32 = mybir.dt.float32
F32R = mybir.dt.float32r
F16 = mybir.dt.float16

USE_F32R = os.environ.get("KERNEL_NO_F32R", "") == ""
# dtype used for all matmul operands on the heavy path. float32r is the PE's
# fast fp32 feed mode (4x faster, mantissa rounded to 11 bits); producers must
# write f32r-typed tiles so walrus sees rounded inputs.
DT_MM = F32R if USE_F32R else F32


def round_f32r(a):
    """Emulate walrus fp32->fp32r rounding (round-half-up at mantissa bit 12)."""
    if not USE_F32R:
        return np.ascontiguousarray(a, np.float32)
    u = np.ascontiguousarray(a, np.float32).view(np.uint32)
    r = ((u.astype(np.uint64) + 0x800) & 0xFFFFF000).astype(np.uint32)
    return r.view(np.float32)


def build_nc(skip_s2d=False, skip_pool=False, skip_exp=False, ntiles=NT):
    nc = bacc.Bacc("TRN2", target_bir_lowering=False)

    # ---- per-core I/O ----
    hT = nc.dram_tensor("hT", [NT, P, KT, RT], F16, kind="ExternalInput")
    wfc = nc.dram_tensor("wfc", [P, KT, HID], F16, kind="ExternalInput")
    wa = nc.dram_tensor("wa", [P, MH, DATT], DT_MM, kind="ExternalInput")
    wb = nc.dram_tensor("wb", [P, MH, DATT], DT_MM, kind="ExternalInput")
    wc = nc.dram_tensor("wc", [P, MD], DT_MM, kind="ExternalInput")
    bfc = nc.dram_tensor("bfc", [P, MH], F32, kind="ExternalInput")
    ba = nc.dram_tensor("ba", [P, MD], F32, kind="ExternalInput")
    bb = nc.dram_tensor("bb", [P, MD], F32, kind="ExternalInput")
    maskt = nc.dram_tensor("maskt", [1, RT], F32, kind="ExternalInput")
    bcr = nc.dram_tensor("bcr", [1, 1], F32, kind="ExternalInput")
    negc = nc.dram_tensor("negc", [1, 1], F32, kind="ExternalInput")

    o_s = nc.dram_tensor("o_s", [1, R], F32, kind="ExternalOutput")
    o_p = nc.dram_tensor("o_p", [P, MH], F32, kind="ExternalOutput")
    o_z = nc.dram_tensor("o_z", [1, 1], F32, kind="ExternalOutput")


    AF = mybir.ActivationFunctionType

    with TileContext(nc) as tc:
        with (
            tc.tile_pool(name="singles", bufs=1) as singles,
            tc.tile_pool(name="hpool", bufs=4) as hpool,
            tc.tile_pool(name="acts", bufs=3) as acts,
            tc.tile_pool(name="rows", bufs=4) as rows,
            tc.tile_pool(name="ps_x", bufs=2, space="PSUM") as ps_x,
            tc.tile_pool(name="ps_att", bufs=2, space="PSUM") as ps_att,
            tc.tile_pool(name="ps_row", bufs=2, space="PSUM") as ps_row,
            tc.tile_pool(name="ps_misc", bufs=1, space="PSUM") as ps_misc,
        ):
            # ---- resident tiles (wfc + first h tile first: they gate PE) ----
            wfc_sb = singles.tile([P, KT, HID], F16)
            nc.sync.dma_start(wfc_sb, wfc[:, :, :])
            ht0 = hpool.tile([P, KT, RT], F16, tag="ht")
            nc.sync.dma_start(ht0, hT[0, :, :, :])
            bfc_sb0 = None
            wa_sb = singles.tile([P, MH, DATT], DT_MM)
            nc.sync.dma_start(wa_sb, wa[:, :, :])
            wb_sb = singles.tile([P, MH, DATT], DT_MM)
            nc.sync.dma_start(wb_sb, wb[:, :, :])
            wc_sb = singles.tile([P, MD], DT_MM)
            nc.sync.dma_start(wc_sb, wc[:, :])
            bfc_sb = singles.tile([P, MH], F32)
            nc.sync.dma_start(bfc_sb, bfc[:, :])
            ba_sb = singles.tile([P, MD], F32)
            nc.sync.dma_start(ba_sb, ba[:, :])
            bb_sb = singles.tile([P, MD], F32)
            nc.sync.dma_start(bb_sb, bb[:, :])
            maskt_sb = singles.tile([1, RT], F32)
            nc.sync.dma_start(maskt_sb, maskt[:, :])
            bcr_sb = singles.tile([1, 1], F32)
            nc.sync.dma_start(bcr_sb, bcr[:, :])
            negc_sb = singles.tile([1, 1], F32)
            nc.sync.dma_start(negc_sb, negc[:, :])

            s_all = singles.tile([1, R], F32)
            zparts = singles.tile([1, NT], F32)
            pacc = singles.tile([P, MH], F32)
            pparts = singles.tile([P, MH, NT], F32)

            for t in range(ntiles):
                r0 = t * RT
                rt = min(RT, R - r0)

                if t == 0:
                    ht = ht0
                else:
                    ht = hpool.tile([P, KT, RT], F16, tag="ht")
                    nc.sync.dma_start(ht[:, :, :rt], hT[t, :, :, :rt])

                # fc + relu -> xT [P, MH, rt]
                xt = acts.tile([P, MH, RT], DT_MM, tag="xt")
                for m in range(MH):
                    px = ps_x.tile([P, RT], F32, tag="px")
                    for k in range(KT):
                        nc.tensor.matmul(
                            px[:, :rt],
                            lhsT=wfc_sb[:, k, m * P : (m + 1) * P],
                            rhs=ht[:, k, :rt],
                            start=(k == 0),
                            stop=(k == KT - 1),
                        )
                    nc.scalar.activation(
                        xt[:, m, :rt], px[:, :rt], AF.Relu,
                        bias=bfc_sb[:, m : m + 1],
                    )

                # gated attention: tanh / tanh-half branches -> ag [P, MD, rt]
                at = acts.tile([P, MD, RT], F32, tag="at")
                gt = acts.tile([P, MD, RT], F32, tag="gt")
                for d in range(MD):
                    pa = ps_att.tile([P, RT], F32, tag="pa")
                    for m in range(MH):
                        nc.tensor.matmul(
                            pa[:, :rt],
                            lhsT=wa_sb[:, m, d * P : (d + 1) * P],
                            rhs=xt[:, m, :rt],
                            start=(m == 0),
                            stop=(m == MH - 1),
                        )
                    nc.scalar.activation(
                        at[:, d, :rt], pa[:, :rt], AF.Tanh,
                        bias=ba_sb[:, d : d + 1],
                    )
                for d in range(MD):
                    pg = ps_att.tile([P, RT], F32, tag="pg")
                    for m in range(MH):
                        nc.tensor.matmul(
                            pg[:, :rt],
                            lhsT=wb_sb[:, m, d * P : (d + 1) * P],
                            rhs=xt[:, m, :rt],
                            start=(m == 0),
                            stop=(m == MH - 1),
                        )
                    # sigmoid(y) = (tanh(y/2) + 1) / 2; the 1/2 is folded into
                    # Wc host-side, so gt holds tanh(y/2) and ag = (gt+1)*at.
                    nc.scalar.activation(
                        gt[:, d, :rt], pg[:, :rt], AF.Tanh,
                        bias=bb_sb[:, d : d + 1], scale=0.5,
                    )
                ag = acts.tile([P, MD, RT], DT_MM, tag="ag")
                nc.vector.scalar_tensor_tensor(
                    out=ag[:, :, :rt],
                    in0=gt[:, :, :rt],
                    scalar=1.0,
                    in1=at[:, :, :rt],
                    op0=mybir.AluOpType.add,
                    op1=mybir.AluOpType.mult,
                )

                # scores: s = (a*g) @ Wc.T  (psum [1, rt])
                ps = ps_row.tile([1, RT], F32, tag="ps")
                for d in range(MD):
                    nc.tensor.matmul(
                        ps[:, :rt],
                        lhsT=wc_sb[:, d : d + 1],
                        rhs=ag[:, d, :rt],
                        start=(d == 0),
                        stop=(d == MD - 1),
                    )

                # raw scores (with bc) accumulated into the output row (DVE)
                nc.vector.tensor_scalar_add(
                    s_all[:, r0 : r0 + rt], ps[:, :rt], bcr_sb[:, :]
                )
                # e = exp(s + bc - C), z partial via accumulate output.
                # Only the last tile can contain padded rows; it gets the
                # additive -1e30 mask first.
                if t == ntiles - 1:
                    s_m = rows.tile([1, RT], F32, tag="s_m")
                    nc.vector.tensor_add(
                        s_m[:, :rt], ps[:, :rt], maskt_sb[:, :rt]
                    )
                    exp_in = s_m
                else:
                    exp_in = ps
                e_row = rows.tile([1, RT], DT_MM, tag="e_row")
                if skip_exp:
                    nc.vector.memset(e_row[:, :rt].bitcast(F32), 0.0)
                    nc.vector.memset(zparts[:, t : t + 1], 0.0)
                else:
                    nc.scalar.activation(
                        e_row[:, :rt], exp_in[:, :rt], AF.Exp,
                        bias=negc_sb[:, :],
                        accum_out=zparts[:, t : t + 1],
                    )

                # broadcast e across partitions on the idle GpSimd engine
                if not skip_pool:
                    peb = acts.tile([P, RT], F32, tag="peb")
                    nc.gpsimd.partition_broadcast(
                        peb[:, :rt], e_row[:, :rt].bitcast(F32)
                    )
                    # pooled partials: pparts[:, m, t] = sum_r xT * e
                    trash = acts.tile([P, RT], F32, tag="trash")
                    for m in range(MH):
                        nc.vector.scalar_tensor_tensor(
                            out=trash[:, :rt],
                            in0=xt[:, m, :rt].bitcast(F32),
                            scalar=1.0,
                            in1=peb[:, :rt],
                            op0=mybir.AluOpType.mult,
                            op1=mybir.AluOpType.mult,
                            accum_out=pparts[:, m, t : t + 1],
                        )
                elif t == 0:
                    nc.vector.memset(pacc[:, :], 0.0)
                    nc.vector.memset(pparts[:, :, :], 0.0)


            # final small reductions + writeback
            if ntiles < NT:
                nc.vector.memset(zparts[:, ntiles:], 0.0)
                nc.vector.memset(pparts[:, :, ntiles:], 0.0)
            if not skip_pool:
                nc.vector.tensor_reduce(
                    pacc, pparts, axis=mybir.AxisListType.X, op=mybir.AluOpType.add
                )
            zsum = singles.tile([1, 1], F32)
            nc.vector.tensor_reduce(
                zsum, zparts, axis=mybir.AxisListType.X, op=mybir.AluOpType.add
            )
            nc.sync.dma_start(o_s[:, :], s_all)
            nc.sync.dma_start(o_p[:, :], pacc)
            nc.sync.dma_start(o_z[:, :], zsum)

    nc.compile()
    return nc


_NC_CACHE = {}


def _get_nc():
    key = USE_F32R
    if key not in _NC_CACHE:
        _NC_CACHE[key] = build_nc()
    return _NC_CACHE[key]


def _prep_inputs(h, W_fc, b_fc, Wa, ba, Wb, bb, Wc, bc):
    """Host-side layout prep -> list of 8 per-core input dicts."""
    f = np.float32
    h = np.asarray(h, f)
    W_fc = np.asarray(W_fc, f)
    Wa = np.asarray(Wa, f)
    Wb = np.asarray(Wb, f)
    Wc = np.asarray(Wc, f)
    b_fc = np.asarray(b_fc, f)
    ba = np.asarray(ba, f)
    bb = np.asarray(bb, f)
    bc = np.asarray(bc, f)

    wfc_in = np.ascontiguousarray(W_fc.reshape(HID, KT, P).transpose(2, 1, 0)).astype(np.float16)
    wa_in = round_f32r(Wa.reshape(DATT, MH, P).transpose(2, 1, 0))
    wb_in = round_f32r(Wb.reshape(DATT, MH, P).transpose(2, 1, 0))
    wc_in = round_f32r(0.5 * Wc.reshape(1, MD, P)[0].transpose(1, 0))
    bfc_in = np.ascontiguousarray(b_fc.reshape(MH, P).T)
    ba_in = np.ascontiguousarray(ba.reshape(MD, P).T)
    bb_in = np.ascontiguousarray(bb.reshape(MD, P).T) * np.float32(0.5)

    # fixed softmax shift: guaranteed upper bound on |s|
    c_shift = float(np.abs(Wc).sum() + np.abs(bc).sum() + 1.0)
    negc_in = np.full((1, 1), float(bc[0]) - c_shift, f)
    bcr_in = np.full((1, 1), float(bc[0]), f)

    in_maps = []
    for c in range(N_CORES):
        hs = h[c * RV : (c + 1) * RV]
        hs_pad = np.zeros((R, FEAT), f)
        hs_pad[:RV] = hs
        # [NT, P, KT, RT]: per (tile, partition) the 8x512 block is contiguous
        hT = np.ascontiguousarray(
            hs_pad.reshape(NT, RT, KT, P).transpose(0, 3, 2, 1)
        ).astype(np.float16)

        mask = np.zeros((1, RT), f)
        last_start = (NT - 1) * RT
        if RV < R:
            lo = max(RV - last_start, 0)
            mask[0, lo:] = NEG_BIG

        in_maps.append(
            {
                "hT": hT,
                "wfc": wfc_in,
                "wa": wa_in,
                "wb": wb_in,
                "wc": wc_in,
                "bfc": bfc_in,
                "ba": ba_in,
                "bb": bb_in,
                "maskt": mask,
                "bcr": bcr_in,
                "negc": negc_in,
            }
        )
    return in_maps, c_shift


def _host_head(results, h, label, W_fc, b_fc, W_cls, b_cls, W_inst, b_inst):
    """Merge per-core partials and evaluate the tiny classifier head."""
    f = np.float32
    W_fc = np.asarray(W_fc, f)
    b_fc = np.asarray(b_fc, f)
    W_cls = np.asarray(W_cls, f)
    b_cls = np.asarray(b_cls, f)
    W_inst = np.asarray(W_inst, f)
    b_inst = np.asarray(b_inst, f)

    s_parts = [results[c]["o_s"][0, :RV] for c in range(N_CORES)]
    s_full = np.concatenate(s_parts)  # [100000]

    z = f(0.0)
    pvec = np.zeros(HID, f)
    for c in range(N_CORES):
        z = z + results[c]["o_z"][0, 0]
        pvec = pvec + results[c]["o_p"].T.reshape(-1)

    M = (pvec / z).astype(f)[None, :]  # [1, HID]

    logits = (M @ W_cls.T + b_cls).astype(f)  # [1, 2]
    ex = np.exp(logits - logits.max(axis=1, keepdims=True))
    Y_prob = (ex / ex.sum(axis=1, keepdims=True)).astype(f)
    Y_hat = np.argmax(logits, axis=1).astype(np.int32)[:, None]

    # top/bottom-k instance selection (stable sort matches jax tie-break)
    top_ids = np.argsort(-s_full, kind="stable")[:K_SAMPLE]
    bot_ids = np.argsort(s_full, kind="stable")[:K_SAMPLE]
    sel = np.concatenate([top_ids, bot_ids])
    h_sel = np.asarray(h, f)[sel]  # [16, FEAT]
    x_sel = np.maximum(h_sel @ W_fc.T + b_fc, 0.0).astype(f)

    targets = np.concatenate(
        [np.ones(K_SAMPLE, np.int32), np.zeros(K_SAMPLE, np.int32)]
    )
    onehot_t = np.eye(N_CLASSES, dtype=f)[targets]  # [16, 2]
    label_i = int(np.asarray(label))
    inst_onehot = np.eye(N_CLASSES, dtype=f)[label_i]

    total = f(0.0)
    for i in range(N_CLASSES):
        if inst_onehot[i] == 0.0:
            continue
        li = (x_sel @ W_inst[i].T + b_inst[i]).astype(f)  # [16, 2]
        delta = (ALPHA * (1.0 - onehot_t)).astype(f)
        zz = (li + delta) / TAU
        m = zz.max(axis=1, keepdims=True)
        lse = (TAU * (np.log(np.exp(zz - m).sum(axis=1, keepdims=True)) + m))[:, 0]
        s_y = np.take_along_axis(li, targets[:, None], axis=1)[:, 0]
        total = total + inst_onehot[i] * np.mean(lse - s_y)

    return (
        logits.astype(f),
        Y_prob.astype(f),
        Y_hat,
        np.float32(total),
    )


def kernel(h, label, W_fc, b_fc, Wa, ba, Wb, bb, Wc, bc, W_cls, b_cls, W_inst,
           b_inst, _trace=False):
    in_maps, _ = _prep_inputs(h, W_fc, b_fc, Wa, ba, Wb, bb, Wc, bc)
    nc = _get_nc()
    res = run_bass_kernel_spmd(
        nc, in_maps, core_ids=list(range(N_CORES)), trace=_trace
    )
    out = _host_head(
        res.results, h, label, W_fc, b_fc, W_cls, b_cls, W_inst, b_inst
    )
    if _trace:
        return out, res
    return out


# revision 28
# speedup vs baseline: 1.0085x; 1.0085x over previous
"""Trainium2 Bass kernel for CLAM_SB gated-attention MIL forward pass.

Distribution: instance dim N=100000 is sharded across 8 NeuronCores
(12500 rows each, padded to 12544 = 98*128). Each core computes its
shard of  x = relu(h @ W_fc.T + b);  a = tanh(x@Wa.T);  g = sig(x@Wb.T);
s = (a*g) @ Wc.T + bc  plus partial softmax stats  z = sum(exp(s-C)),
p = sum(exp(s-C) * x)  using a fixed host-computed shift C (softmax is
shift invariant, so no cross-core max pass is needed). The raw score
matrix (tiny, 50KB/core) plus (z, p) are returned; the host merges the
partial stats, does the 64->16 top/bottom-k merge, and evaluates the
final ~10K-FLOP classifier head.
"""

import os
import sys

import numpy as np

try:
    import concourse.bass as _probe_bass  # noqa: F401
except ImportError:
    sys.path.insert(0, "/opt/trn_rl_repo")

import concourse.bass as bass
from concourse import bacc
import concourse.mybir as mybir
from concourse.tile import TileContext
from concourse.bass_utils import run_bass_kernel_spmd

# problem constants (hardcoded per harness contract)
N_CORES = 8
N_FULL = 100000
FEAT = 1024
HID = 256
DATT = 256
K_SAMPLE = 8
TAU = 1.0
ALPHA = 1.0
N_CLASSES = 2

RV = N_FULL // N_CORES          # valid rows per core = 12500
P = 128
RT = 512                        # rows per compute tile
NT = 25                         # uniform tiles per core
R = NT * RT                     # padded rows per core = 12800
NCOL = R // P                   # 100 columns in (p, j) score layout
KT = FEAT // P                  # 8 contraction chunks for fc
MH = HID // P                   # 2 hid chunks
MD = DATT // P                  # 2 attention-dim chunks

NEG_BIG = -1.0e30

F32 = mybir.dt.float32
F32R = mybir.dt.float32r
F16 = mybir.dt.float16

USE_F32R = os.environ.get("KERNEL_NO_F32R", "") == ""
# dtype used for all matmul operands on the heavy path. float32r is the PE's
# fast fp32 feed mode (4x faster, mantissa rounded to 11 bits); producers must
# write f32r-typed tiles so walrus sees rounded inputs.
DT_MM = F32R if USE_F32R else F32


def round_f32r(a):
    """Emulate walrus fp32->fp32r rounding (round-half-up at mantissa bit 12)."""
    if not USE_F32R:
        return np.ascontiguousarray(a, np.float32)
    u = np.ascontiguousarray(a, np.float32).view(np.uint32)
    r = ((u.astype(np.uint64) + 0x800) & 0xFFFFF000).astype(np.uint32)
    return r.view(np.float32)


def build_nc(skip_s2d=False, skip_pool=False, skip_exp=False, ntiles=NT):
    nc = bacc.Bacc("TRN2", target_bir_lowering=False)

    # ---- per-core I/O ----
    hT = nc.dram_tensor("hT", [NT, P, KT, RT], F16, kind="ExternalInput")
    wfc = nc.dram_tensor("wfc", [P, KT, HID], F16, kind="ExternalInput")
    wa = nc.dram_tensor("wa", [P, MH, DATT], DT_MM, kind="ExternalInput")
    wb = nc.dram_tensor("wb", [P, MH, DATT], DT_MM, kind="ExternalInput")
    wc = nc.dram_tensor("wc", [P, MD], DT_MM, kind="ExternalInput")
    bfc = nc.dram_tensor("bfc", [P, MH], F32, kind="ExternalInput")
    ba = nc.dram_tensor("ba", [P, MD], F32, kind="ExternalInput")
    bb = nc.dram_tensor("bb", [P, MD], F32, kind="ExternalInput")
    maskt = nc.dram_tensor("maskt", [1, RT], F32, kind="ExternalInput")
    bcr = nc.dram_tensor("bcr", [1, 1], F32, kind="ExternalInput")
    negc = nc.dram_tensor("negc", [1, 1], F32, kind="ExternalInput")

    o_s = nc.dram_tensor("o_s", [1, R], F32, kind="ExternalOutput")
    o_p = nc.dram_tensor("o_p", [P, MH], F32, kind="ExternalOutput")
    o_z = nc.dram_tensor("o_z", [1, 1], F32, kind="ExternalOutput")


    AF = mybir.ActivationFunctionType

    with TileContext(nc) as tc:
        with (
            tc.tile_pool(name="singles", bufs=1) as singles,
            tc.tile_pool(name="hpool", bufs=4) as hpool,
            tc.tile_pool(name="acts", bufs=3) as acts,
            tc.tile_pool(name="rows", bufs=4) as rows,
            tc.tile_pool(name="ps_x", bufs=2, space="PSUM") as ps_x,
            tc.tile_pool(name="ps_att", bufs=2, space="PSUM") as ps_att,
            tc.tile_pool(name="ps_row", bufs=2, space="PSUM") as ps_row,
            tc.tile_pool(name="ps_misc", bufs=1, space="PSUM") as ps_misc,
        ):
            # ---- resident tiles (wfc + first h tile first: they gate PE) ----
            wfc_sb = singles.tile([P, KT, HID], F16)
            nc.sync.dma_start(wfc_sb, wfc[:, :, :])
            ht0 = hpool.tile([P, KT, RT], F16, tag="ht")
            nc.sync.dma_start(ht0, hT[0, :, :, :])
            bfc_sb0 = None
            wa_sb = singles.tile([P, MH, DATT], DT_MM)
            nc.sync.dma_start(wa_sb, wa[:, :, :])
            wb_sb = singles.tile([P, MH, DATT], DT_MM)
            nc.sync.dma_start(wb_sb, wb[:, :, :])
            wc_sb = singles.tile([P, MD], DT_MM)
            nc.sync.dma_start(wc_sb, wc[:, :])
            bfc_sb = singles.tile([P, MH], F32)
            nc.sync.dma_start(bfc_sb, bfc[:, :])
            ba_sb = singles.tile([P, MD], F32)
            nc.sync.dma_start(ba_sb, ba[:, :])
            bb_sb = singles.tile([P, MD], F32)
            nc.sync.dma_start(bb_sb, bb[:, :])
            maskt_sb = singles.tile([1, RT], F32)
            nc.sync.dma_start(maskt_sb, maskt[:, :])
            bcr_sb = singles.tile([1, 1], F32)
            nc.sync.dma_start(bcr_sb, bcr[:, :])
            negc_sb = singles.tile([1, 1], F32)
            nc.sync.dma_start(negc_sb, negc[:, :])

            s_all = singles.tile([1, R], F32)
            zparts = singles.tile([1, NT], F32)
            pacc = singles.tile([P, MH], F32)
            pparts = singles.tile([P, MH, NT], F32)

            for t in range(ntiles):
                r0 = t * RT
                rt = min(RT, R - r0)

                if t == 0:
                    ht = ht0
                else:
                    ht = hpool.tile([P, KT, RT], F16, tag="ht")
                    nc.sync.dma_start(ht[:, :, :rt], hT[t, :, :, :rt])

                # fc + relu -> xT [P, MH, rt]
                xt = acts.tile([P, MH, RT], DT_MM, tag="xt")
                for m in range(MH):
                    px = ps_x.tile([P, RT], F32, tag="px")
                    for k in range(KT):
                        nc.tensor.matmul(
                            px[:, :rt],
                            lhsT=wfc_sb[:, k, m * P : (m + 1) * P],
                            rhs=ht[:, k, :rt],
                            start=(k == 0),
                            stop=(k == KT - 1),
                        )
                    # relu(psum + b) as one DVE op: (in + b) max 0 — keeps
                    # the fc->att chain off the busier ACT queue
                    nc.vector.tensor_scalar(
                        xt[:, m, :rt], px[:, :rt],
                        bfc_sb[:, m : m + 1], 0.0,
                        op0=mybir.AluOpType.add, op1=mybir.AluOpType.max,
                    )

                # gated attention: tanh / tanh-half branches -> ag [P, MD, rt]
                at = acts.tile([P, MD, RT], F32, tag="at")
                gt = acts.tile([P, MD, RT], F32, tag="gt")
                for d in range(MD):
                    pa = ps_att.tile([P, RT], F32, tag="pa")
                    for m in range(MH):
                        nc.tensor.matmul(
                            pa[:, :rt],
                            lhsT=wa_sb[:, m, d * P : (d + 1) * P],
                            rhs=xt[:, m, :rt],
                            start=(m == 0),
                            stop=(m == MH - 1),
                        )
                    nc.scalar.activation(
                        at[:, d, :rt], pa[:, :rt], AF.Tanh,
                        bias=ba_sb[:, d : d + 1],
                    )
                for d in range(MD):
                    pg = ps_att.tile([P, RT], F32, tag="pg")
                    for m in range(MH):
                        nc.tensor.matmul(
                            pg[:, :rt],
                            lhsT=wb_sb[:, m, d * P : (d + 1) * P],
                            rhs=xt[:, m, :rt],
                            start=(m == 0),
                            stop=(m == MH - 1),
                        )
                    # sigmoid(y) = (tanh(y/2) + 1) / 2; the 1/2 is folded into
                    # Wc host-side, so gt holds tanh(y/2) and ag = (gt+1)*at.
                    nc.scalar.activation(
                        gt[:, d, :rt], pg[:, :rt], AF.Tanh,
                        bias=bb_sb[:, d : d + 1], scale=0.5,
                    )
                ag = acts.tile([P, MD, RT], DT_MM, tag="ag")
                nc.vector.scalar_tensor_tensor(
                    out=ag[:, :, :rt],
                    in0=gt[:, :, :rt],
                    scalar=1.0,
                    in1=at[:, :, :rt],
                    op0=mybir.AluOpType.add,
                    op1=mybir.AluOpType.mult,
                )

                # scores: s = (a*g) @ Wc.T  (psum [1, rt])
                ps = ps_row.tile([1, RT], F32, tag="ps")
                for d in range(MD):
                    nc.tensor.matmul(
                        ps[:, :rt],
                        lhsT=wc_sb[:, d : d + 1],
                        rhs=ag[:, d, :rt],
                        start=(d == 0),
                        stop=(d == MD - 1),
                    )

                # raw scores (with bc) accumulated into the output row (DVE)
                nc.vector.tensor_scalar_add(
                    s_all[:, r0 : r0 + rt], ps[:, :rt], bcr_sb[:, :]
                )
                # e = exp(s + bc - C), z partial via accumulate output.
                # Only the last tile can contain padded rows; it gets the
                # additive -1e30 mask first.
                if t == ntiles - 1:
                    s_m = rows.tile([1, RT], F32, tag="s_m")
                    nc.vector.tensor_add(
                        s_m[:, :rt], ps[:, :rt], maskt_sb[:, :rt]
                    )
                    exp_in = s_m
                else:
                    exp_in = ps
                e_row = rows.tile([1, RT], DT_MM, tag="e_row")
                if skip_exp:
                    nc.vector.memset(e_row[:, :rt].bitcast(F32), 0.0)
                    nc.vector.memset(zparts[:, t : t + 1], 0.0)
                else:
                    nc.scalar.activation(
                        e_row[:, :rt], exp_in[:, :rt], AF.Exp,
                        bias=negc_sb[:, :],
                        accum_out=zparts[:, t : t + 1],
                    )

                # broadcast e across partitions on the idle GpSimd engine
                if not skip_pool:
                    peb = acts.tile([P, RT], F32, tag="peb")
                    nc.gpsimd.partition_broadcast(
                        peb[:, :rt], e_row[:, :rt].bitcast(F32)
                    )
                    # pooled partials: pparts[:, m, t] = sum_r xT * e
                    trash = acts.tile([P, RT], F32, tag="trash")
                    for m in range(MH):
                        nc.vector.scalar_tensor_tensor(
                            out=trash[:, :rt],
                            in0=xt[:, m, :rt].bitcast(F32),
                            scalar=1.0,
                            in1=peb[:, :rt],
                            op0=mybir.AluOpType.mult,
                            op1=mybir.AluOpType.mult,
                            accum_out=pparts[:, m, t : t + 1],
                        )
                elif t == 0:
                    nc.vector.memset(pacc[:, :], 0.0)
                    nc.vector.memset(pparts[:, :, :], 0.0)


            # final small reductions + writeback
            if ntiles < NT:
                nc.vector.memset(zparts[:, ntiles:], 0.0)
                nc.vector.memset(pparts[:, :, ntiles:], 0.0)
            if not skip_pool:
                nc.vector.tensor_reduce(
                    pacc, pparts, axis=mybir.AxisListType.X, op=mybir.AluOpType.add
                )
            zsum = singles.tile([1, 1], F32)
            nc.vector.tensor_reduce(
                zsum, zparts, axis=mybir.AxisListType.X, op=mybir.AluOpType.add
            )
            nc.sync.dma_start(o_s[:, :], s_all)
            nc.sync.dma_start(o_p[:, :], pacc)
            nc.sync.dma_start(o_z[:, :], zsum)

    nc.compile()
    return nc


_NC_CACHE = {}


def _get_nc():
    key = USE_F32R
    if key not in _NC_CACHE:
        _NC_CACHE[key] = build_nc()
    return _NC_CACHE[key]


def _prep_inputs(h, W_fc, b_fc, Wa, ba, Wb, bb, Wc, bc):
    """Host-side layout prep -> list of 8 per-core input dicts."""
    f = np.float32
    h = np.asarray(h, f)
    W_fc = np.asarray(W_fc, f)
    Wa = np.asarray(Wa, f)
    Wb = np.asarray(Wb, f)
    Wc = np.asarray(Wc, f)
    b_fc = np.asarray(b_fc, f)
    ba = np.asarray(ba, f)
    bb = np.asarray(bb, f)
    bc = np.asarray(bc, f)

    wfc_in = np.ascontiguousarray(W_fc.reshape(HID, KT, P).transpose(2, 1, 0)).astype(np.float16)
    wa_in = round_f32r(Wa.reshape(DATT, MH, P).transpose(2, 1, 0))
    wb_in = round_f32r(Wb.reshape(DATT, MH, P).transpose(2, 1, 0))
    wc_in = round_f32r(0.5 * Wc.reshape(1, MD, P)[0].transpose(1, 0))
    bfc_in = np.ascontiguousarray(b_fc.reshape(MH, P).T)
    ba_in = np.ascontiguousarray(ba.reshape(MD, P).T)
    bb_in = np.ascontiguousarray(bb.reshape(MD, P).T) * np.float32(0.5)

    # fixed softmax shift: guaranteed upper bound on |s|
    c_shift = float(np.abs(Wc).sum() + np.abs(bc).sum() + 1.0)
    negc_in = np.full((1, 1), float(bc[0]) - c_shift, f)
    bcr_in = np.full((1, 1), float(bc[0]), f)

    in_maps = []
    for c in range(N_CORES):
        hs = h[c * RV : (c + 1) * RV]
        hs_pad = np.zeros((R, FEAT), f)
        hs_pad[:RV] = hs
        # [NT, P, KT, RT]: per (tile, partition) the 8x512 block is contiguous
        hT = np.ascontiguousarray(
            hs_pad.reshape(NT, RT, KT, P).transpose(0, 3, 2, 1)
        ).astype(np.float16)

        mask = np.zeros((1, RT), f)
        last_start = (NT - 1) * RT
        if RV < R:
            lo = max(RV - last_start, 0)
            mask[0, lo:] = NEG_BIG

        in_maps.append(
            {
                "hT": hT,
                "wfc": wfc_in,
                "wa": wa_in,
                "wb": wb_in,
                "wc": wc_in,
                "bfc": bfc_in,
                "ba": ba_in,
                "bb": bb_in,
                "maskt": mask,
                "bcr": bcr_in,
                "negc": negc_in,
            }
        )
    return in_maps, c_shift


def _host_head(results, h, label, W_fc, b_fc, W_cls, b_cls, W_inst, b_inst):
    """Merge per-core partials and evaluate the tiny classifier head."""
    f = np.float32
    W_fc = np.asarray(W_fc, f)
    b_fc = np.asarray(b_fc, f)
    W_cls = np.asarray(W_cls, f)
    b_cls = np.asarray(b_cls, f)
    W_inst = np.asarray(W_inst, f)
    b_inst = np.asarray(b_inst, f)

    s_parts = [results[c]["o_s"][0, :RV] for c in range(N_CORES)]
    s_full = np.concatenate(s_parts)  # [100000]

    z = f(0.0)
    pvec = np.zeros(HID, f)
    for c in range(N_CORES):
        z = z + results[c]["o_z"][0, 0]
        pvec = pvec + results[c]["o_p"].T.reshape(-1)

    M = (pvec / z).astype(f)[None, :]  # [1, HID]

    logits = (M @ W_cls.T + b_cls).astype(f)  # [1, 2]
    ex = np.exp(logits - logits.max(axis=1, keepdims=True))
    Y_prob = (ex / ex.sum(axis=1, keepdims=True)).astype(f)
    Y_hat = np.argmax(logits, axis=1).astype(np.int32)[:, None]

    # top/bottom-k instance selection (stable sort matches jax tie-break)
    top_ids = np.argsort(-s_full, kind="stable")[:K_SAMPLE]
    bot_ids = np.argsort(s_full, kind="stable")[:K_SAMPLE]
    sel = np.concatenate([top_ids, bot_ids])
    h_sel = np.asarray(h, f)[sel]  # [16, FEAT]
    x_sel = np.maximum(h_sel @ W_fc.T + b_fc, 0.0).astype(f)

    targets = np.concatenate(
        [np.ones(K_SAMPLE, np.int32), np.zeros(K_SAMPLE, np.int32)]
    )
    onehot_t = np.eye(N_CLASSES, dtype=f)[targets]  # [16, 2]
    label_i = int(np.asarray(label))
    inst_onehot = np.eye(N_CLASSES, dtype=f)[label_i]

    total = f(0.0)
    for i in range(N_CLASSES):
        if inst_onehot[i] == 0.0:
            continue
        li = (x_sel @ W_inst[i].T + b_inst[i]).astype(f)  # [16, 2]
        delta = (ALPHA * (1.0 - onehot_t)).astype(f)
        zz = (li + delta) / TAU
        m = zz.max(axis=1, keepdims=True)
        lse = (TAU * (np.log(np.exp(zz - m).sum(axis=1, keepdims=True)) + m))[:, 0]
        s_y = np.take_along_axis(li, targets[:, None], axis=1)[:, 0]
        total = total + inst_onehot[i] * np.mean(lse - s_y)

    return (
        logits.astype(f),
        Y_prob.astype(f),
        Y_hat,
        np.float32(total),
    )


def kernel(h, label, W_fc, b_fc, Wa, ba, Wb, bb, Wc, bc, W_cls, b_cls, W_inst,
           b_inst, _trace=False):
    in_maps, _ = _prep_inputs(h, W_fc, b_fc, Wa, ba, Wb, bb, Wc, bc)
    nc = _get_nc()
    res = run_bass_kernel_spmd(
        nc, in_maps, core_ids=list(range(N_CORES)), trace=_trace
    )
    out = _host_head(
        res.results, h, label, W_fc, b_fc, W_cls, b_cls, W_inst, b_inst
    )
    if _trace:
        return out, res
    return out
